# revision 17
# baseline (speedup 1.0000x reference)
"""MoE LoRA adapter layer (top-2 routed, E=8 experts, R=16) on 8 TRN2 NeuronCores.

Strategy: data-parallel over batch B=32 -> 4 batches/core; router + LoRA
weights replicated (tiny). E*R = 128 = partition width, so the per-expert
LoRA down/up projections stack into two dense matmuls:
    P1T[er, t] = D_all[er, :] @ x[t, :]^T          (contract H=1024)
    wT[h, t]   = U_all[er, h]^T @ (gate * P1T)     (contract ER=128)
The expert sum IS the matmul contraction; gates (exactly 0 off the top-2)
are folded in by scaling P1T columns per batch.

Everything runs in the transposed domain so the PE never transposes:
x is shipped pre-transposed from the host as xt[p, half, k, t] (bf16) and
y is stored transposed as y_out[k, p, t], un-transposed on the host. The
residual add yT = wT + xT reuses the same xt tiles MM1 consumed, so HBM
traffic stays at the 8.4 MiB minimum (4 MiB load + 4 MiB store + weights).
Loads ride the sync HWDGE ring; weights + stores ride the scalar HWDGE
ring. Gates are computed on-device in fp32 (exact top-2).
"""

import sys

if "/opt/trn_rl_repo" not in sys.path:
    sys.path.insert(0, "/opt/trn_rl_repo")

import numpy as np
import ml_dtypes

import concourse.bass as bass
import concourse.tile as tile
from concourse import bacc, mybir
from concourse.bass_utils import run_bass_kernel_spmd

B, L, H = 32, 512, 1024
E, R, TOP_K = 8, 16, 2
N_CORES = 8
NB = B // N_CORES          # batches per core = 4
T = NB * L                 # tokens per core = 2048
P = 128                    # partitions
NK = H // P                # H k-tiles = 8
NS = 2                     # halves (2 batches = 1024 tokens each)
HT = T // NS               # tokens per half = 1024
NQ = 2                     # k-quads per half-load (4 k-tiles per DMA)

F32 = mybir.dt.float32
BF16 = mybir.dt.bfloat16
BF16_NP = ml_dtypes.bfloat16

_COMPILED = None


def _build():
    """Build + compile the single-core program (same on all 8 cores)."""
    nc = bacc.Bacc("TRN2", target_bir_lowering=False, debug=False)

    xt_in = nc.dram_tensor("xt_in", [P, NS * NK * HT], BF16, kind="ExternalInput")
    clsT_in = nc.dram_tensor("clsT_in", [P, NK * NB], F32, kind="ExternalInput")
    d_t = nc.dram_tensor("d_t", [P, NK * P], BF16, kind="ExternalInput")
    u_in = nc.dram_tensor("u_in", [P, H], BF16, kind="ExternalInput")
    rwt = nc.dram_tensor("rwt", [P, NK * E], F32, kind="ExternalInput")
    rep = nc.dram_tensor("rep", [E, P], F32, kind="ExternalInput")
    idn = nc.dram_tensor("idn", [P, P], F32, kind="ExternalInput")
    idnb = nc.dram_tensor("idnb", [P, P], BF16, kind="ExternalInput")
    y_out = nc.dram_tensor("y_out", [NK, P * T], BF16, kind="ExternalOutput")

    # y_out[k, (p s t)] -> [s, p, k, t]
    y_view = y_out.ap().rearrange("k (p s t) -> s p k t", p=P, s=NS, t=HT)

    with tile.TileContext(nc) as tc:
        with (
            tc.tile_pool(name="wpool", bufs=1) as wpool,
            tc.tile_pool(name="xpool", bufs=1) as xpool,
            tc.tile_pool(name="ypool", bufs=1) as ypool,
            tc.tile_pool(name="p2pool", bufs=2) as p2pool,
            tc.tile_pool(name="gpool", bufs=1) as gpool,
            tc.tile_pool(name="p1_ps", bufs=1, space="PSUM") as p1_ps,
            tc.tile_pool(name="w_ps", bufs=3, space="PSUM") as w_ps,
        ):
            # ---- scalar ring: gates inputs first (fast HWDGE), then d/u.
            # sync ring: x s0 loads, then stores. gpsimd ring: x s1 loads. ----
            clsT = gpool.tile([P, NK * NB], F32, tag="clsT")
            nc.scalar.dma_start(clsT[:], clsT_in.ap())
            rwt_sb = wpool.tile([P, NK * E], F32, tag="rwt")
            nc.scalar.dma_start(rwt_sb[:], rwt.ap())
            id_sb = wpool.tile([P, P], F32, tag="idn")
            nc.scalar.dma_start(id_sb[:], idn.ap())
            rep_sb = wpool.tile([E, P], F32, tag="rep")
            nc.scalar.dma_start(rep_sb[:], rep.ap())

            xt = xpool.tile([P, NS, NK, HT], BF16, tag="xt")
            x_kview = xt_in.ap().rearrange(
                "p (s k t) -> s p k t", s=NS, k=NK, t=HT
            )
            for kp in range(0, NK, 2):
                nc.sync.dma_start(
                    xt[:, 0, kp : kp + 2, :], x_kview[0][:, kp : kp + 2, :]
                )
            for kp in range(0, NK, 2):
                nc.gpsimd.dma_start(
                    xt[:, 1, kp : kp + 2, :], x_kview[1][:, kp : kp + 2, :]
                )

            d_sb = wpool.tile([P, NK * P], BF16, tag="d")
            nc.scalar.dma_start(d_sb[:], d_t.ap())
            u_sb = wpool.tile([P, H], BF16, tag="u")
            nc.scalar.dma_start(u_sb[:], u_in.ap())
            idb_sb = wpool.tile([P, P], BF16, tag="idnb")
            nc.scalar.dma_start(idb_sb[:], idnb.ap())

            yt = ypool.tile([P, NK, T], BF16, tag="yt")

            # ---- gates prologue (fp32, exact top-2; clsT pre-transposed) ----
            lg_ps = w_ps.tile([P, 512], F32, tag="w")
            for k in range(NK):
                nc.tensor.matmul(
                    lg_ps[0:NB, 0:E],
                    clsT[:, k * NB : (k + 1) * NB],
                    rwt_sb[:, k * E : (k + 1) * E],
                    start=(k == 0),
                    stop=(k == NK - 1),
                )
            lg = gpool.tile([NB, E], F32, tag="lg")
            nc.vector.tensor_copy(lg[:], lg_ps[0:NB, 0:E])

            # top-2 softmax per row (E=8 along free dim)
            m1 = gpool.tile([NB, 1], F32, tag="m1")
            nc.vector.reduce_max(m1[:], lg[:], axis=mybir.AxisListType.X)
            t_sb = gpool.tile([NB, E], F32, tag="t")
            nc.vector.tensor_scalar(
                t_sb[:], lg[:], m1[:], None, op0=mybir.AluOpType.subtract
            )
            # pen = (t >= 0) * 1e30  (knocks out the argmax)
            pen = gpool.tile([NB, E], F32, tag="pen")
            nc.vector.tensor_scalar(
                pen[:], t_sb[:], 0.0, 1e30,
                op0=mybir.AluOpType.is_ge, op1=mybir.AluOpType.mult,
            )
            t2 = gpool.tile([NB, E], F32, tag="t2")
            nc.vector.tensor_sub(t2[:], t_sb[:], pen[:])
            m2 = gpool.tile([NB, 1], F32, tag="m2")
            nc.vector.reduce_max(m2[:], t2[:], axis=mybir.AxisListType.X)
            keep = gpool.tile([NB, E], F32, tag="keep")
            nc.vector.tensor_scalar(
                keep[:], t_sb[:], m2[:], None, op0=mybir.AluOpType.is_ge
            )
            ex = gpool.tile([NB, E], F32, tag="ex")
            nc.scalar.activation(ex[:], t_sb[:], mybir.ActivationFunctionType.Exp)
            eg = gpool.tile([NB, E], F32, tag="eg")
            nc.vector.tensor_mul(eg[:], ex[:], keep[:])
            s_sb = gpool.tile([NB, 1], F32, tag="s")
            nc.vector.reduce_sum(s_sb[:], eg[:], axis=mybir.AxisListType.X)
            rs = gpool.tile([NB, 1], F32, tag="rs")
            nc.vector.reciprocal(rs[:], s_sb[:])
            gts = gpool.tile([NB, E], F32, tag="gts")
            nc.vector.tensor_scalar(
                gts[:], eg[:], rs[:], None, op0=mybir.AluOpType.mult
            )

            # gatesT then replicate x16 along partitions -> gvec [128, NB]
            gt_ps = w_ps.tile([P, 512], F32, tag="w")
            nc.tensor.transpose(gt_ps[0:E, 0:NB], gts[:], id_sb[0:NB, 0:NB])
            gtT = gpool.tile([E, NB], F32, tag="gtT")
            nc.vector.tensor_copy(gtT[:], gt_ps[0:E, 0:NB])
            gv_ps = w_ps.tile([P, 512], F32, tag="w")
            nc.tensor.matmul(gv_ps[:, 0:NB], rep_sb[:], gtT[:])
            gvec = gpool.tile([P, NB], F32, tag="gvec")
            nc.vector.tensor_copy(gvec[:], gv_ps[:, 0:NB])

            # ---- main: per half (2 batches = 1024 tokens) ----
            for s in range(NS):
                p1 = p1_ps.tile([P, HT], F32, tag="p1")
                for k in range(NK):
                    for hc in range(0, HT, 512):
                        nc.tensor.matmul(
                            p1[:, hc : hc + 512],
                            d_sb[:, k * P : (k + 1) * P],
                            xt[:, s, k, hc : hc + 512],
                            start=(k == 0),
                            stop=(k == NK - 1),
                        )
                p2t = p2pool.tile([P, HT], BF16, tag="p2t")
                for b2 in range(2):
                    c = s * 2 + b2
                    nc.scalar.activation(
                        p2t[:, b2 * L : (b2 + 1) * L],
                        p1[:, b2 * L : (b2 + 1) * L],
                        mybir.ActivationFunctionType.Copy,
                        scale=gvec[:, c : c + 1],
                    )
                for k in range(NK):
                    wps = w_ps.tile([P, HT], F32, tag="w")
                    pe_res = k % 2 == 1  # odd k: residual add rides PE + ACT
                    for hc in range(0, HT, 512):
                        nc.tensor.matmul(
                            wps[:, hc : hc + 512],
                            u_sb[:, k * P : (k + 1) * P],
                            p2t[:, hc : hc + 512],
                            start=True,
                            stop=not pe_res,
                        )
                    ys = yt[:, k, s * HT : (s + 1) * HT]
                    if pe_res:
                        for hc in range(0, HT, 512):
                            nc.tensor.matmul(
                                wps[:, hc : hc + 512],
                                idb_sb[:],
                                xt[:, s, k, hc : hc + 512],
                                start=False,
                                stop=True,
                            )
                        nc.scalar.activation(
                            ys, wps[:], mybir.ActivationFunctionType.Copy
                        )
                    else:
                        nc.vector.tensor_add(ys, wps[:], xt[:, s, k, :])
                    if k % 2 == 1:
                        nc.sync.dma_start(
                            y_view[s][:, k - 1 : k + 1, :],
                            yt[:, k - 1 : k + 1, s * HT : (s + 1) * HT],
                        )

    nc.compile()
    return nc


def _weights_maps(router_w, lora_down, lora_up):
    # D_all[(e,r), h] stacked; lhsT tiles need [p, k, m] = D_all[m, k*128+p]
    d_all = lora_down.reshape(E * R, H)                       # [128, 1024]
    d_t = np.ascontiguousarray(
        d_all.T.reshape(NK, P, E * R).transpose(1, 0, 2).reshape(P, NK * P)
    ).astype(BF16_NP)
    # U_all[(e,r), h] = lora_up[e, h, r]
    u_np = np.ascontiguousarray(
        lora_up.transpose(0, 2, 1).reshape(E * R, H)
    ).astype(BF16_NP)
    # router_wT tiles [p, k, e] = router_w[e, k*128+p]
    rwt_np = np.ascontiguousarray(
        router_w.T.reshape(NK, P, E).transpose(1, 0, 2).reshape(P, NK * E)
    ).astype(np.float32)
    rep_np = np.zeros((E, P), np.float32)
    for e in range(E):
        rep_np[e, e * R : (e + 1) * R] = 1.0
    idn_np = np.eye(P, dtype=np.float32)
    return {
        "d_t": d_t, "u_in": u_np, "rwt": rwt_np, "rep": rep_np,
        "idn": idn_np, "idnb": idn_np.astype(BF16_NP),
    }


def get_compiled():
    global _COMPILED
    if _COMPILED is None:
        _COMPILED = _build()
    return _COMPILED


def make_in_maps(x, router_w, lora_down, lora_up):
    x = np.asarray(x, np.float32)
    w_maps = _weights_maps(
        np.asarray(router_w, np.float32),
        np.asarray(lora_down, np.float32),
        np.asarray(lora_up, np.float32),
    )
    in_maps = []
    for i in range(N_CORES):
        xc = x[i * NB : (i + 1) * NB].reshape(T, H)
        # xt[p, (s k t)] = x[s*HT + t, k*128 + p]
        xt = np.ascontiguousarray(
            xc.reshape(NS, HT, NK, P).transpose(3, 0, 2, 1).reshape(P, NS * NK * HT)
        ).astype(BF16_NP)
        cls = x[i * NB : (i + 1) * NB, 0, :]
        # clsT[p, (k b)] = cls[b, k*128 + p]
        clsT = np.ascontiguousarray(
            cls.T.reshape(NK, P, NB).transpose(1, 0, 2).reshape(P, NK * NB)
        ).astype(np.float32)
        in_maps.append({"xt_in": xt, "clsT_in": clsT, **w_maps})
    return in_maps


def unshard_core(y_np):
    """y_out [NK, P*T] (bf16) -> [NB, L, H] f32."""
    y = np.asarray(y_np, np.float32).reshape(NK, P, T)
    return y.transpose(2, 0, 1).reshape(NB, L, H)


def kernel(x, router_w, lora_down, lora_up):
    nc = get_compiled()
    in_maps = make_in_maps(x, router_w, lora_down, lora_up)
    res = run_bass_kernel_spmd(nc, in_maps, core_ids=list(range(N_CORES)))
    out = np.empty((B, L, H), np.float32)
    for i in range(N_CORES):
        out[i * NB : (i + 1) * NB] = unshard_core(res.results[i]["y_out"])
    return out


# revision 18
# speedup vs baseline: 1.1581x; 1.1581x over previous
"""MoE LoRA adapter layer (top-2 routed, E=8 experts, R=16) on 8 TRN2 NeuronCores.

Strategy: data-parallel over batch B=32 -> 4 batches/core; router + LoRA
weights replicated (tiny). E*R = 128 = partition width, so the per-expert
LoRA down/up projections stack into two dense matmuls:
    P1T[er, t] = D_all[er, :] @ x[t, :]^T          (contract H=1024)
    wT[h, t]   = U_all[er, h]^T @ (gate * P1T)     (contract ER=128)
The expert sum IS the matmul contraction; gates (exactly 0 off the top-2)
are folded in by scaling P1T columns per batch (ACT copy with per-partition
scale).

Everything runs in the transposed domain so the PE never transposes:
x is shipped pre-transposed from the host as xt[p, c, k, t] (bf16, c = batch
quarter) and y is stored transposed as y_out[k, p, t], un-transposed on the
host. The residual add yT = wT + xT (DVE) reuses the same xt tiles MM1
consumed, so HBM traffic stays at the ~8.4 MiB minimum per core.

Pipeline granularity is one batch (512 tokens): MM1 (8 mms) -> gate-scale
(ACT) -> MM2 (8 mms, 1-bank PSUM tiles, 5 rotating buffers) -> DVE adds ->
k-quad stores. Gates inputs ride one packed DMA at the head of the sync
ring; x loads follow (8 x 512 KB); stores trail on the same ring. d/u
weights ride the scalar ring concurrently.
"""

import sys

if "/opt/trn_rl_repo" not in sys.path:
    sys.path.insert(0, "/opt/trn_rl_repo")

import numpy as np
import ml_dtypes

import concourse.bass as bass
import concourse.tile as tile
from concourse import bacc, mybir
from concourse.bass_utils import run_bass_kernel_spmd

B, L, H = 32, 512, 1024
E, R, TOP_K = 8, 16, 2
N_CORES = 8
NB = B // N_CORES          # batches (quarters) per core = 4
T = NB * L                 # tokens per core = 2048
P = 128                    # partitions
NK = H // P                # H k-tiles = 8

F32 = mybir.dt.float32
BF16 = mybir.dt.bfloat16
BF16_NP = ml_dtypes.bfloat16

# packed gates-input layout: [clsT | rwt | idn | rep] along free dim (f32)
G_CLS = 0
G_RWT = G_CLS + NK * NB        # 32
G_IDN = G_RWT + NK * E         # 96
G_REP = G_IDN + P              # 224
G_END = G_REP + P              # 352

_COMPILED = None


def _build():
    """Build + compile the single-core program (same on all 8 cores)."""
    nc = bacc.Bacc("TRN2", target_bir_lowering=False, debug=False)

    xt_in = nc.dram_tensor("xt_in", [P, NB * NK * L], BF16, kind="ExternalInput")
    gpk_in = nc.dram_tensor("gpk_in", [P, G_END], F32, kind="ExternalInput")
    d_t = nc.dram_tensor("d_t", [P, NK * P], BF16, kind="ExternalInput")
    u_in = nc.dram_tensor("u_in", [P, H], BF16, kind="ExternalInput")
    y_out = nc.dram_tensor("y_out", [NK, P * T], BF16, kind="ExternalOutput")

    # y_out[k, (p c t)] -> [c, p, k, t]
    y_view = y_out.ap().rearrange("k (p c t) -> c p k t", p=P, c=NB, t=L)

    with tile.TileContext(nc) as tc:
        with (
            tc.tile_pool(name="wpool", bufs=1) as wpool,
            tc.tile_pool(name="xpool", bufs=1) as xpool,
            tc.tile_pool(name="ypool", bufs=1) as ypool,
            tc.tile_pool(name="p2pool", bufs=3) as p2pool,
            tc.tile_pool(name="gpool", bufs=1) as gpool,
            tc.tile_pool(name="p1_ps", bufs=2, space="PSUM") as p1_ps,
            tc.tile_pool(name="w_ps", bufs=5, space="PSUM") as w_ps,
        ):
            # ---- sync ring: packed gates inputs, then x (8 x 512 KB), then
            # stores. scalar ring: d/u weights (concurrent queue). ----
            gpk = wpool.tile([P, G_END], F32, tag="gpk")
            nc.sync.dma_start(gpk[:], gpk_in.ap())
            clsT = gpk[:, G_CLS:G_RWT]
            rwt_sb = gpk[:, G_RWT:G_IDN]
            id_sb = gpk[:, G_IDN:G_REP]
            rep_sb = gpk[0:E, G_REP:G_END]

            xt = xpool.tile([P, NB, NK, L], BF16, tag="xt")
            x_kview = xt_in.ap().rearrange(
                "p (c k t) -> c p k t", c=NB, k=NK, t=L
            )
            for c in range(NB):
                for kh in range(2):
                    nc.sync.dma_start(
                        xt[:, c, kh * 4 : (kh + 1) * 4, :],
                        x_kview[c][:, kh * 4 : (kh + 1) * 4, :],
                    )

            d_sb = wpool.tile([P, NK * P], BF16, tag="d")
            nc.scalar.dma_start(d_sb[:], d_t.ap())
            u_sb = wpool.tile([P, H], BF16, tag="u")
            nc.scalar.dma_start(u_sb[:], u_in.ap())

            yt = ypool.tile([P, NK, T], BF16, tag="yt")

            # ---- gates prologue (fp32, exact top-2; clsT pre-transposed) ----
            lg_ps = w_ps.tile([P, 512], F32, tag="w")
            for k in range(NK):
                nc.tensor.matmul(
                    lg_ps[0:NB, 0:E],
                    clsT[:, k * NB : (k + 1) * NB],
                    rwt_sb[:, k * E : (k + 1) * E],
                    start=(k == 0),
                    stop=(k == NK - 1),
                )
            lg = gpool.tile([NB, E], F32, tag="lg")
            nc.vector.tensor_copy(lg[:], lg_ps[0:NB, 0:E])

            # top-2 softmax per row (E=8 along free dim)
            m1 = gpool.tile([NB, 1], F32, tag="m1")
            nc.vector.reduce_max(m1[:], lg[:], axis=mybir.AxisListType.X)
            t_sb = gpool.tile([NB, E], F32, tag="t")
            nc.vector.tensor_scalar(
                t_sb[:], lg[:], m1[:], None, op0=mybir.AluOpType.subtract
            )
            # pen = (t >= 0) * 1e30  (knocks out the argmax)
            pen = gpool.tile([NB, E], F32, tag="pen")
            nc.vector.tensor_scalar(
                pen[:], t_sb[:], 0.0, 1e30,
                op0=mybir.AluOpType.is_ge, op1=mybir.AluOpType.mult,
            )
            t2 = gpool.tile([NB, E], F32, tag="t2")
            nc.vector.tensor_sub(t2[:], t_sb[:], pen[:])
            m2 = gpool.tile([NB, 1], F32, tag="m2")
            nc.vector.reduce_max(m2[:], t2[:], axis=mybir.AxisListType.X)
            keep = gpool.tile([NB, E], F32, tag="keep")
            nc.vector.tensor_scalar(
                keep[:], t_sb[:], m2[:], None, op0=mybir.AluOpType.is_ge
            )
            ex = gpool.tile([NB, E], F32, tag="ex")
            nc.scalar.activation(ex[:], t_sb[:], mybir.ActivationFunctionType.Exp)
            eg = gpool.tile([NB, E], F32, tag="eg")
            nc.vector.tensor_mul(eg[:], ex[:], keep[:])
            s_sb = gpool.tile([NB, 1], F32, tag="s")
            nc.vector.reduce_sum(s_sb[:], eg[:], axis=mybir.AxisListType.X)
            rs = gpool.tile([NB, 1], F32, tag="rs")
            nc.vector.reciprocal(rs[:], s_sb[:])
            gts = gpool.tile([NB, E], F32, tag="gts")
            nc.vector.tensor_scalar(
                gts[:], eg[:], rs[:], None, op0=mybir.AluOpType.mult
            )

            # gatesT then replicate x16 along partitions -> gvec [128, NB]
            gt_ps = w_ps.tile([P, 512], F32, tag="w")
            nc.tensor.transpose(gt_ps[0:E, 0:NB], gts[:], id_sb[0:NB, 0:NB])
            gtT = gpool.tile([E, NB], F32, tag="gtT")
            nc.vector.tensor_copy(gtT[:], gt_ps[0:E, 0:NB])
            gv_ps = w_ps.tile([P, 512], F32, tag="w")
            nc.tensor.matmul(gv_ps[:, 0:NB], rep_sb[:], gtT[:])
            gvec = gpool.tile([P, NB], F32, tag="gvec")
            nc.vector.tensor_copy(gvec[:], gv_ps[:, 0:NB])

            # ---- main: per quarter (1 batch = 512 tokens) ----
            for c in range(NB):
                p1 = p1_ps.tile([P, L], F32, tag="p1")
                for k in range(NK):
                    nc.tensor.matmul(
                        p1[:],
                        d_sb[:, k * P : (k + 1) * P],
                        xt[:, c, k, :],
                        start=(k == 0),
                        stop=(k == NK - 1),
                    )
                p2t = p2pool.tile([P, L], BF16, tag="p2t")
                nc.scalar.activation(
                    p2t[:], p1[:],
                    mybir.ActivationFunctionType.Copy,
                    scale=gvec[:, c : c + 1],
                )
                for k in range(NK):
                    wps = w_ps.tile([P, L], F32, tag="w")
                    nc.tensor.matmul(
                        wps[:], u_sb[:, k * P : (k + 1) * P], p2t[:]
                    )
                    nc.vector.tensor_add(
                        yt[:, k, c * L : (c + 1) * L], wps[:], xt[:, c, k, :]
                    )
                    if k % 4 == 3:
                        nc.sync.dma_start(
                            y_view[c][:, k - 3 : k + 1, :],
                            yt[:, k - 3 : k + 1, c * L : (c + 1) * L],
                        )

    nc.compile()
    return nc


def _weights_maps(router_w, lora_down, lora_up):
    # D_all[(e,r), h] stacked; lhsT tiles need [p, k, m] = D_all[m, k*128+p]
    d_all = lora_down.reshape(E * R, H)                       # [128, 1024]
    d_t = np.ascontiguousarray(
        d_all.T.reshape(NK, P, E * R).transpose(1, 0, 2).reshape(P, NK * P)
    ).astype(BF16_NP)
    # U_all[(e,r), h] = lora_up[e, h, r]
    u_np = np.ascontiguousarray(
        lora_up.transpose(0, 2, 1).reshape(E * R, H)
    ).astype(BF16_NP)
    # router_wT tiles [p, k, e] = router_w[e, k*128+p]
    rwt_np = np.ascontiguousarray(
        router_w.T.reshape(NK, P, E).transpose(1, 0, 2).reshape(P, NK * E)
    ).astype(np.float32)
    rep_np = np.zeros((P, P), np.float32)
    for e in range(E):
        rep_np[e, e * R : (e + 1) * R] = 1.0
    idn_np = np.eye(P, dtype=np.float32)
    return {"d_t": d_t, "u_in": u_np, "rwt": rwt_np,
            "rep": rep_np, "idn": idn_np}


def get_compiled():
    global _COMPILED
    if _COMPILED is None:
        _COMPILED = _build()
    return _COMPILED


def make_in_maps(x, router_w, lora_down, lora_up):
    x = np.asarray(x, np.float32)
    w = _weights_maps(
        np.asarray(router_w, np.float32),
        np.asarray(lora_down, np.float32),
        np.asarray(lora_up, np.float32),
    )
    in_maps = []
    for i in range(N_CORES):
        xc = x[i * NB : (i + 1) * NB].reshape(T, H)
        # xt[p, (c k t)] = x[c*L + t, k*128 + p]
        xt = np.ascontiguousarray(
            xc.reshape(NB, L, NK, P).transpose(3, 0, 2, 1).reshape(P, NB * NK * L)
        ).astype(BF16_NP)
        cls = x[i * NB : (i + 1) * NB, 0, :]
        # clsT[p, (k b)] = cls[b, k*128 + p]
        clsT = np.ascontiguousarray(
            cls.T.reshape(NK, P, NB).transpose(1, 0, 2).reshape(P, NK * NB)
        ).astype(np.float32)
        gpk = np.concatenate([clsT, w["rwt"], w["idn"], w["rep"]], axis=1)
        in_maps.append({
            "xt_in": xt,
            "gpk_in": np.ascontiguousarray(gpk),
            "d_t": w["d_t"],
            "u_in": w["u_in"],
        })
    return in_maps


def unshard_core(y_np):
    """y_out [NK, P*T] (bf16) -> [NB, L, H] f32."""
    y = np.asarray(y_np, np.float32).reshape(NK, P, T)
    return y.transpose(2, 0, 1).reshape(NB, L, H)


def kernel(x, router_w, lora_down, lora_up):
    nc = get_compiled()
    in_maps = make_in_maps(x, router_w, lora_down, lora_up)
    res = run_bass_kernel_spmd(nc, in_maps, core_ids=list(range(N_CORES)))
    out = np.empty((B, L, H), np.float32)
    for i in range(N_CORES):
        out[i * NB : (i + 1) * NB] = unshard_core(res.results[i]["y_out"])
    return out


# revision 23
# speedup vs baseline: 1.1838x; 1.0222x over previous
"""MoE LoRA adapter layer (top-2 routed, E=8 experts, R=16) on 8 TRN2 NeuronCores.

Strategy: data-parallel over batch B=32 -> 4 batches/core; router + LoRA
weights replicated (tiny). E*R = 128 = partition width, so the per-expert
LoRA down/up projections stack into two dense matmuls:
    P1T[er, t] = D_all[er, :] @ x[t, :]^T          (contract H=1024)
    wT[h, t]   = U_all[er, h]^T @ (gate * P1T)     (contract ER=128)
The expert sum IS the matmul contraction; gates (exactly 0 off the top-2)
are folded in by scaling P1T columns per batch (ACT copy with per-partition
scale).

Everything runs in the transposed domain so the PE never transposes:
x is shipped pre-transposed from the host as xt[p, c, k, t] (bf16, c = batch
quarter) and y is stored transposed as y_out[k, p, t], un-transposed on the
host. The residual add yT = wT + xT (DVE) reuses the same xt tiles MM1
consumed, so HBM traffic stays at the ~8.4 MiB minimum per core.

Pipeline granularity is one batch (512 tokens): MM1 (8 mms) -> gate-scale
(ACT) -> MM2 (8 mms, 1-bank PSUM tiles, 5 rotating buffers) -> DVE adds ->
k-quad stores. Gates inputs ride one packed DMA at the head of the sync
ring; x loads follow (8 x 512 KB); stores trail on the same ring. d/u
weights ride the scalar ring concurrently.
"""

import sys

if "/opt/trn_rl_repo" not in sys.path:
    sys.path.insert(0, "/opt/trn_rl_repo")

import numpy as np
import ml_dtypes

import concourse.bass as bass
import concourse.tile as tile
from concourse import bacc, mybir
from concourse.bass_utils import run_bass_kernel_spmd

B, L, H = 32, 512, 1024
E, R, TOP_K = 8, 16, 2
N_CORES = 8
NB = B // N_CORES          # batches (quarters) per core = 4
T = NB * L                 # tokens per core = 2048
P = 128                    # partitions
NK = H // P                # H k-tiles = 8

F32 = mybir.dt.float32
BF16 = mybir.dt.bfloat16
BF16_NP = ml_dtypes.bfloat16

# packed gates-input layout: [clsT | rwt | idn | rep] along free dim (f32)
G_CLS = 0
G_RWT = G_CLS + NK * NB        # 32
G_IDN = G_RWT + NK * E         # 96
G_REP = G_IDN + P              # 224
G_END = G_REP + P              # 352

_COMPILED = None


def _build():
    """Build + compile the single-core program (same on all 8 cores)."""
    nc = bacc.Bacc("TRN2", target_bir_lowering=False, debug=False)

    xt_in = nc.dram_tensor("xt_in", [P, NB * NK * L], BF16, kind="ExternalInput")
    gpk_in = nc.dram_tensor("gpk_in", [P, G_END], F32, kind="ExternalInput")
    d_t = nc.dram_tensor("d_t", [P, NK * P], BF16, kind="ExternalInput")
    u_in = nc.dram_tensor("u_in", [P, H], BF16, kind="ExternalInput")
    idnb = nc.dram_tensor("idnb", [P, P], BF16, kind="ExternalInput")
    y_out = nc.dram_tensor("y_out", [NK, P * T], BF16, kind="ExternalOutput")

    # y_out[k, (p c t)] -> [c, p, k, t]
    y_view = y_out.ap().rearrange("k (p c t) -> c p k t", p=P, c=NB, t=L)

    with tile.TileContext(nc) as tc:
        with (
            tc.tile_pool(name="wpool", bufs=1) as wpool,
            tc.tile_pool(name="xpool", bufs=1) as xpool,
            tc.tile_pool(name="ypool", bufs=1) as ypool,
            tc.tile_pool(name="p2pool", bufs=3) as p2pool,
            tc.tile_pool(name="gpool", bufs=1) as gpool,
            tc.tile_pool(name="p1_ps", bufs=2, space="PSUM") as p1_ps,
            tc.tile_pool(name="w_ps", bufs=3, space="PSUM") as w_ps,
        ):
            # ---- sync ring: x loads (8 x 512 KB) then stores. scalar ring:
            # packed gates inputs + d/u weights (concurrent queue). ----
            xt = xpool.tile([P, NB, NK, L], BF16, tag="xt")
            x_kview = xt_in.ap().rearrange(
                "p (c k t) -> c p k t", c=NB, k=NK, t=L
            )
            for c in range(NB):
                for kh in range(2):
                    nc.sync.dma_start(
                        xt[:, c, kh * 4 : (kh + 1) * 4, :],
                        x_kview[c][:, kh * 4 : (kh + 1) * 4, :],
                    )

            gpk = wpool.tile([P, G_END], F32, tag="gpk")
            nc.scalar.dma_start(gpk[:], gpk_in.ap())
            clsT = gpk[:, G_CLS:G_RWT]
            rwt_sb = gpk[:, G_RWT:G_IDN]
            id_sb = gpk[:, G_IDN:G_REP]
            rep_sb = gpk[0:E, G_REP:G_END]

            d_sb = wpool.tile([P, NK * P], BF16, tag="d")
            nc.scalar.dma_start(d_sb[:], d_t.ap())
            u_sb = wpool.tile([P, H], BF16, tag="u")
            nc.scalar.dma_start(u_sb[:], u_in.ap())
            idb_sb = wpool.tile([P, P], BF16, tag="idnb")
            nc.scalar.dma_start(idb_sb[:], idnb.ap())

            yt = ypool.tile([P, NK, T], BF16, tag="yt")

            # ---- gates prologue (fp32, exact top-2; clsT pre-transposed) ----
            lg_ps = w_ps.tile([P, 512], F32, tag="w")
            for k in range(NK):
                nc.tensor.matmul(
                    lg_ps[0:NB, 0:E],
                    clsT[:, k * NB : (k + 1) * NB],
                    rwt_sb[:, k * E : (k + 1) * E],
                    start=(k == 0),
                    stop=(k == NK - 1),
                )
            lg = gpool.tile([NB, E], F32, tag="lg")
            nc.vector.tensor_copy(lg[:], lg_ps[0:NB, 0:E])

            # top-2 softmax per row (E=8 along free dim)
            m1 = gpool.tile([NB, 1], F32, tag="m1")
            nc.vector.reduce_max(m1[:], lg[:], axis=mybir.AxisListType.X)
            t_sb = gpool.tile([NB, E], F32, tag="t")
            nc.vector.tensor_scalar(
                t_sb[:], lg[:], m1[:], None, op0=mybir.AluOpType.subtract
            )
            # pen = (t >= 0) * 1e30  (knocks out the argmax)
            pen = gpool.tile([NB, E], F32, tag="pen")
            nc.vector.tensor_scalar(
                pen[:], t_sb[:], 0.0, 1e30,
                op0=mybir.AluOpType.is_ge, op1=mybir.AluOpType.mult,
            )
            t2 = gpool.tile([NB, E], F32, tag="t2")
            nc.vector.tensor_sub(t2[:], t_sb[:], pen[:])
            m2 = gpool.tile([NB, 1], F32, tag="m2")
            nc.vector.reduce_max(m2[:], t2[:], axis=mybir.AxisListType.X)
            keep = gpool.tile([NB, E], F32, tag="keep")
            nc.vector.tensor_scalar(
                keep[:], t_sb[:], m2[:], None, op0=mybir.AluOpType.is_ge
            )
            ex = gpool.tile([NB, E], F32, tag="ex")
            nc.scalar.activation(ex[:], t_sb[:], mybir.ActivationFunctionType.Exp)
            eg = gpool.tile([NB, E], F32, tag="eg")
            nc.vector.tensor_mul(eg[:], ex[:], keep[:])
            s_sb = gpool.tile([NB, 1], F32, tag="s")
            nc.vector.reduce_sum(s_sb[:], eg[:], axis=mybir.AxisListType.X)
            rs = gpool.tile([NB, 1], F32, tag="rs")
            nc.vector.reciprocal(rs[:], s_sb[:])
            gts = gpool.tile([NB, E], F32, tag="gts")
            nc.vector.tensor_scalar(
                gts[:], eg[:], rs[:], None, op0=mybir.AluOpType.mult
            )

            # gatesT then replicate x16 along partitions -> gvec [128, NB]
            gt_ps = w_ps.tile([P, 512], F32, tag="w")
            nc.tensor.transpose(gt_ps[0:E, 0:NB], gts[:], id_sb[0:NB, 0:NB])
            gtT = gpool.tile([E, NB], F32, tag="gtT")
            nc.vector.tensor_copy(gtT[:], gt_ps[0:E, 0:NB])
            gv_ps = w_ps.tile([P, 512], F32, tag="w")
            nc.tensor.matmul(gv_ps[:, 0:NB], rep_sb[:], gtT[:])
            gvec = gpool.tile([P, NB], F32, tag="gvec")
            nc.vector.tensor_copy(gvec[:], gv_ps[:, 0:NB])

            # ---- main: per quarter (1 batch = 512 tokens) ----
            for c in range(NB):
                p1 = p1_ps.tile([P, L], F32, tag="p1")
                for k in range(NK):
                    nc.tensor.matmul(
                        p1[:],
                        d_sb[:, k * P : (k + 1) * P],
                        xt[:, c, k, :],
                        start=(k == 0),
                        stop=(k == NK - 1),
                    )
                p2t = p2pool.tile([P, L], BF16, tag="p2t")
                nc.scalar.activation(
                    p2t[:], p1[:],
                    mybir.ActivationFunctionType.Copy,
                    scale=gvec[:, c : c + 1],
                )
                for kp in range(NK // 2):
                    k0 = 2 * kp
                    wps = w_ps.tile([P, 2, L], F32, tag="w")
                    act_pair = kp % 2 == 0  # residual via PE + ACT copy
                    for kk in range(2):
                        k = k0 + kk
                        nc.tensor.matmul(
                            wps[:, kk, :],
                            u_sb[:, k * P : (k + 1) * P],
                            p2t[:],
                            start=True,
                            stop=not act_pair,
                        )
                        if act_pair:
                            nc.tensor.matmul(
                                wps[:, kk, :],
                                idb_sb[:],
                                xt[:, c, k, :],
                                start=False,
                                stop=True,
                            )
                    ys = yt[:, k0 : k0 + 2, c * L : (c + 1) * L]
                    if act_pair:
                        nc.scalar.activation(
                            ys, wps[:], mybir.ActivationFunctionType.Copy
                        )
                    else:
                        nc.vector.tensor_add(
                            ys, wps[:], xt[:, c, k0 : k0 + 2, :]
                        )
                    if kp % 2 == 1:
                        nc.sync.dma_start(
                            y_view[c][:, k0 - 2 : k0 + 2, :],
                            yt[:, k0 - 2 : k0 + 2, c * L : (c + 1) * L],
                        )

    nc.compile()
    return nc


def _weights_maps(router_w, lora_down, lora_up):
    # D_all[(e,r), h] stacked; lhsT tiles need [p, k, m] = D_all[m, k*128+p]
    d_all = lora_down.reshape(E * R, H)                       # [128, 1024]
    d_t = np.ascontiguousarray(
        d_all.T.reshape(NK, P, E * R).transpose(1, 0, 2).reshape(P, NK * P)
    ).astype(BF16_NP)
    # U_all[(e,r), h] = lora_up[e, h, r]
    u_np = np.ascontiguousarray(
        lora_up.transpose(0, 2, 1).reshape(E * R, H)
    ).astype(BF16_NP)
    # router_wT tiles [p, k, e] = router_w[e, k*128+p]
    rwt_np = np.ascontiguousarray(
        router_w.T.reshape(NK, P, E).transpose(1, 0, 2).reshape(P, NK * E)
    ).astype(np.float32)
    rep_np = np.zeros((P, P), np.float32)
    for e in range(E):
        rep_np[e, e * R : (e + 1) * R] = 1.0
    idn_np = np.eye(P, dtype=np.float32)
    return {"d_t": d_t, "u_in": u_np, "rwt": rwt_np,
            "rep": rep_np, "idn": idn_np,
            "idnb": idn_np.astype(BF16_NP)}


def get_compiled():
    global _COMPILED
    if _COMPILED is None:
        _COMPILED = _build()
    return _COMPILED


def make_in_maps(x, router_w, lora_down, lora_up):
    x = np.asarray(x, np.float32)
    w = _weights_maps(
        np.asarray(router_w, np.float32),
        np.asarray(lora_down, np.float32),
        np.asarray(lora_up, np.float32),
    )
    in_maps = []
    for i in range(N_CORES):
        xc = x[i * NB : (i + 1) * NB].reshape(T, H)
        # xt[p, (c k t)] = x[c*L + t, k*128 + p]
        xt = np.ascontiguousarray(
            xc.reshape(NB, L, NK, P).transpose(3, 0, 2, 1).reshape(P, NB * NK * L)
        ).astype(BF16_NP)
        cls = x[i * NB : (i + 1) * NB, 0, :]
        # clsT[p, (k b)] = cls[b, k*128 + p]
        clsT = np.ascontiguousarray(
            cls.T.reshape(NK, P, NB).transpose(1, 0, 2).reshape(P, NK * NB)
        ).astype(np.float32)
        gpk = np.concatenate([clsT, w["rwt"], w["idn"], w["rep"]], axis=1)
        in_maps.append({
            "xt_in": xt,
            "gpk_in": np.ascontiguousarray(gpk),
            "d_t": w["d_t"],
            "u_in": w["u_in"],
            "idnb": w["idnb"],
        })
    return in_maps


def unshard_core(y_np):
    """y_out [NK, P*T] (bf16) -> [NB, L, H] f32."""
    y = np.asarray(y_np, np.float32).reshape(NK, P, T)
    return y.transpose(2, 0, 1).reshape(NB, L, H)


def kernel(x, router_w, lora_down, lora_up):
    nc = get_compiled()
    in_maps = make_in_maps(x, router_w, lora_down, lora_up)
    res = run_bass_kernel_spmd(nc, in_maps, core_ids=list(range(N_CORES)))
    out = np.empty((B, L, H), np.float32)
    for i in range(N_CORES):
        out[i * NB : (i + 1) * NB] = unshard_core(res.results[i]["y_out"])
    return out


# revision 25
# speedup vs baseline: 1.2158x; 1.0270x over previous
"""MoE LoRA adapter layer (top-2 routed, E=8 experts, R=16) on 8 TRN2 NeuronCores.

Strategy: data-parallel over batch B=32 -> 4 batches/core; router + LoRA
weights replicated (tiny). E*R = 128 = partition width, so the per-expert
LoRA down/up projections stack into two dense matmuls:
    P1T[er, t] = D_all[er, :] @ x[t, :]^T          (contract H=1024)
    wT[h, t]   = U_all[er, h]^T @ (gate * P1T)     (contract ER=128)
The expert sum IS the matmul contraction; gates (exactly 0 off the top-2)
are folded in by scaling P1T columns per batch (ACT copy with per-partition
scale).

Everything runs in the transposed domain so the PE never transposes:
x is shipped pre-transposed from the host as xt[p, c, k, t] (bf16, c = batch
quarter) and y is stored transposed as y_out[k, p, t], un-transposed on the
host. The residual add yT = wT + xT (DVE) reuses the same xt tiles MM1
consumed, so HBM traffic stays at the ~8.4 MiB minimum per core.

Pipeline granularity is one batch (512 tokens): MM1 (8 mms) -> gate-scale
(ACT) -> MM2 (8 mms, 1-bank PSUM tiles, 5 rotating buffers) -> DVE adds ->
k-quad stores. Gates inputs ride one packed DMA at the head of the sync
ring; x loads follow (8 x 512 KB); stores trail on the same ring. d/u
weights ride the scalar ring concurrently.
"""

import sys

if "/opt/trn_rl_repo" not in sys.path:
    sys.path.insert(0, "/opt/trn_rl_repo")

import numpy as np
import ml_dtypes

import concourse.bass as bass
import concourse.tile as tile
from concourse import bacc, mybir
from concourse.bass_utils import run_bass_kernel_spmd

B, L, H = 32, 512, 1024
E, R, TOP_K = 8, 16, 2
N_CORES = 8
NB = B // N_CORES          # batches (quarters) per core = 4
T = NB * L                 # tokens per core = 2048
P = 128                    # partitions
NK = H // P                # H k-tiles = 8

F32 = mybir.dt.float32
BF16 = mybir.dt.bfloat16
BF16_NP = ml_dtypes.bfloat16

# packed gates-input layout: [clsT | rwt | idn | rep] along free dim (f32)
G_CLS = 0
G_RWT = G_CLS + NK * NB        # 32
G_IDN = G_RWT + NK * E         # 96
G_REP = G_IDN + P              # 224
G_END = G_REP + P              # 352

_COMPILED = None


def _build():
    """Build + compile the single-core program (same on all 8 cores)."""
    nc = bacc.Bacc("TRN2", target_bir_lowering=False, debug=False)

    xt_in = nc.dram_tensor("xt_in", [P, NB * NK * L], BF16, kind="ExternalInput")
    gpk_in = nc.dram_tensor("gpk_in", [P, G_END], F32, kind="ExternalInput")
    d_t = nc.dram_tensor("d_t", [P, NK * P], BF16, kind="ExternalInput")
    u_in = nc.dram_tensor("u_in", [P, H], BF16, kind="ExternalInput")
    idnb = nc.dram_tensor("idnb", [P, P], BF16, kind="ExternalInput")
    y_out = nc.dram_tensor("y_out", [NK, P * T], BF16, kind="ExternalOutput")

    # y_out[k, (p c t)] -> [c, p, k, t]
    y_view = y_out.ap().rearrange("k (p c t) -> c p k t", p=P, c=NB, t=L)

    with tile.TileContext(nc) as tc:
        with (
            tc.tile_pool(name="wpool", bufs=1) as wpool,
            tc.tile_pool(name="xpool", bufs=1) as xpool,
            tc.tile_pool(name="ypool", bufs=1) as ypool,
            tc.tile_pool(name="p2pool", bufs=3) as p2pool,
            tc.tile_pool(name="gpool", bufs=1) as gpool,
            tc.tile_pool(name="p1_ps", bufs=2, space="PSUM") as p1_ps,
            tc.tile_pool(name="w_ps", bufs=3, space="PSUM") as w_ps,
        ):
            # ---- single sync ring, ordered so all 8 DMA sem lanes pair
            # benignly: gpk, d, u, idnb, x (c0 split small), stores later.
            # Scalar ring carries no DMAs (ACT compute stays unblocked). ----
            gpk = wpool.tile([P, G_END], F32, tag="gpk")
            nc.sync.dma_start(gpk[:], gpk_in.ap())
            clsT = gpk[:, G_CLS:G_RWT]
            rwt_sb = gpk[:, G_RWT:G_IDN]
            id_sb = gpk[:, G_IDN:G_REP]
            rep_sb = gpk[0:E, G_REP:G_END]

            d_sb = wpool.tile([P, NK * P], BF16, tag="d")
            nc.sync.dma_start(d_sb[:], d_t.ap())
            u_sb = wpool.tile([P, H], BF16, tag="u")
            nc.sync.dma_start(u_sb[:], u_in.ap())
            idb_sb = wpool.tile([P, P], BF16, tag="idnb")
            nc.sync.dma_start(idb_sb[:], idnb.ap())

            xt = xpool.tile([P, NB, NK, L], BF16, tag="xt")
            x_kview = xt_in.ap().rearrange(
                "p (c k t) -> c p k t", c=NB, k=NK, t=L
            )
            nc.sync.dma_start(xt[:, 0, 0:4, :], x_kview[0][:, 0:4, :])
            nc.sync.dma_start(xt[:, 0, 4:8, :], x_kview[0][:, 4:8, :])
            for c in range(1, NB):
                nc.sync.dma_start(xt[:, c], x_kview[c])

            yt = ypool.tile([P, NK, T], BF16, tag="yt")

            # ---- gates prologue (fp32, exact top-2; clsT pre-transposed) ----
            lg_ps = w_ps.tile([P, 512], F32, tag="w")
            for k in range(NK):
                nc.tensor.matmul(
                    lg_ps[0:NB, 0:E],
                    clsT[:, k * NB : (k + 1) * NB],
                    rwt_sb[:, k * E : (k + 1) * E],
                    start=(k == 0),
                    stop=(k == NK - 1),
                )
            lg = gpool.tile([NB, E], F32, tag="lg")
            nc.vector.tensor_copy(lg[:], lg_ps[0:NB, 0:E])

            # top-2 softmax per row (E=8 along free dim)
            m1 = gpool.tile([NB, 1], F32, tag="m1")
            nc.vector.reduce_max(m1[:], lg[:], axis=mybir.AxisListType.X)
            t_sb = gpool.tile([NB, E], F32, tag="t")
            nc.vector.tensor_scalar(
                t_sb[:], lg[:], m1[:], None, op0=mybir.AluOpType.subtract
            )
            # pen = (t >= 0) * 1e30  (knocks out the argmax)
            pen = gpool.tile([NB, E], F32, tag="pen")
            nc.vector.tensor_scalar(
                pen[:], t_sb[:], 0.0, 1e30,
                op0=mybir.AluOpType.is_ge, op1=mybir.AluOpType.mult,
            )
            t2 = gpool.tile([NB, E], F32, tag="t2")
            nc.vector.tensor_sub(t2[:], t_sb[:], pen[:])
            m2 = gpool.tile([NB, 1], F32, tag="m2")
            nc.vector.reduce_max(m2[:], t2[:], axis=mybir.AxisListType.X)
            keep = gpool.tile([NB, E], F32, tag="keep")
            nc.vector.tensor_scalar(
                keep[:], t_sb[:], m2[:], None, op0=mybir.AluOpType.is_ge
            )
            ex = gpool.tile([NB, E], F32, tag="ex")
            nc.scalar.activation(ex[:], t_sb[:], mybir.ActivationFunctionType.Exp)
            eg = gpool.tile([NB, E], F32, tag="eg")
            nc.vector.tensor_mul(eg[:], ex[:], keep[:])
            s_sb = gpool.tile([NB, 1], F32, tag="s")
            nc.vector.reduce_sum(s_sb[:], eg[:], axis=mybir.AxisListType.X)
            rs = gpool.tile([NB, 1], F32, tag="rs")
            nc.vector.reciprocal(rs[:], s_sb[:])
            gts = gpool.tile([NB, E], F32, tag="gts")
            nc.vector.tensor_scalar(
                gts[:], eg[:], rs[:], None, op0=mybir.AluOpType.mult
            )

            # gatesT then replicate x16 along partitions -> gvec [128, NB]
            gt_ps = w_ps.tile([P, 512], F32, tag="w")
            nc.tensor.transpose(gt_ps[0:E, 0:NB], gts[:], id_sb[0:NB, 0:NB])
            gtT = gpool.tile([E, NB], F32, tag="gtT")
            nc.vector.tensor_copy(gtT[:], gt_ps[0:E, 0:NB])
            gv_ps = w_ps.tile([P, 512], F32, tag="w")
            nc.tensor.matmul(gv_ps[:, 0:NB], rep_sb[:], gtT[:])
            gvec = gpool.tile([P, NB], F32, tag="gvec")
            nc.vector.tensor_copy(gvec[:], gv_ps[:, 0:NB])

            # ---- main: per quarter (1 batch = 512 tokens) ----
            for c in range(NB):
                p1 = p1_ps.tile([P, L], F32, tag="p1")
                for k in range(NK):
                    nc.tensor.matmul(
                        p1[:],
                        d_sb[:, k * P : (k + 1) * P],
                        xt[:, c, k, :],
                        start=(k == 0),
                        stop=(k == NK - 1),
                    )
                p2t = p2pool.tile([P, L], BF16, tag="p2t")
                nc.scalar.activation(
                    p2t[:], p1[:],
                    mybir.ActivationFunctionType.Copy,
                    scale=gvec[:, c : c + 1],
                )
                for kp in range(NK // 2):
                    k0 = 2 * kp
                    wps = w_ps.tile([P, 2, L], F32, tag="w")
                    act_pair = kp == 1  # residual via PE + ACT copy
                    for kk in range(2):
                        k = k0 + kk
                        nc.tensor.matmul(
                            wps[:, kk, :],
                            u_sb[:, k * P : (k + 1) * P],
                            p2t[:],
                            start=True,
                            stop=not act_pair,
                        )
                        if act_pair:
                            nc.tensor.matmul(
                                wps[:, kk, :],
                                idb_sb[:],
                                xt[:, c, k, :],
                                start=False,
                                stop=True,
                            )
                    ys = yt[:, k0 : k0 + 2, c * L : (c + 1) * L]
                    if act_pair:
                        nc.scalar.activation(
                            ys, wps[:], mybir.ActivationFunctionType.Copy
                        )
                    else:
                        nc.vector.tensor_add(
                            ys, wps[:], xt[:, c, k0 : k0 + 2, :]
                        )
                    if kp % 2 == 1:
                        nc.sync.dma_start(
                            y_view[c][:, k0 - 2 : k0 + 2, :],
                            yt[:, k0 - 2 : k0 + 2, c * L : (c + 1) * L],
                        )

    nc.compile()
    return nc


def _weights_maps(router_w, lora_down, lora_up):
    # D_all[(e,r), h] stacked; lhsT tiles need [p, k, m] = D_all[m, k*128+p]
    d_all = lora_down.reshape(E * R, H)                       # [128, 1024]
    d_t = np.ascontiguousarray(
        d_all.T.reshape(NK, P, E * R).transpose(1, 0, 2).reshape(P, NK * P)
    ).astype(BF16_NP)
    # U_all[(e,r), h] = lora_up[e, h, r]
    u_np = np.ascontiguousarray(
        lora_up.transpose(0, 2, 1).reshape(E * R, H)
    ).astype(BF16_NP)
    # router_wT tiles [p, k, e] = router_w[e, k*128+p]
    rwt_np = np.ascontiguousarray(
        router_w.T.reshape(NK, P, E).transpose(1, 0, 2).reshape(P, NK * E)
    ).astype(np.float32)
    rep_np = np.zeros((P, P), np.float32)
    for e in range(E):
        rep_np[e, e * R : (e + 1) * R] = 1.0
    idn_np = np.eye(P, dtype=np.float32)
    return {"d_t": d_t, "u_in": u_np, "rwt": rwt_np,
            "rep": rep_np, "idn": idn_np,
            "idnb": idn_np.astype(BF16_NP)}


def get_compiled():
    global _COMPILED
    if _COMPILED is None:
        _COMPILED = _build()
    return _COMPILED


def make_in_maps(x, router_w, lora_down, lora_up):
    x = np.asarray(x, np.float32)
    w = _weights_maps(
        np.asarray(router_w, np.float32),
        np.asarray(lora_down, np.float32),
        np.asarray(lora_up, np.float32),
    )
    in_maps = []
    for i in range(N_CORES):
        xc = x[i * NB : (i + 1) * NB].reshape(T, H)
        # xt[p, (c k t)] = x[c*L + t, k*128 + p]
        xt = np.ascontiguousarray(
            xc.reshape(NB, L, NK, P).transpose(3, 0, 2, 1).reshape(P, NB * NK * L)
        ).astype(BF16_NP)
        cls = x[i * NB : (i + 1) * NB, 0, :]
        # clsT[p, (k b)] = cls[b, k*128 + p]
        clsT = np.ascontiguousarray(
            cls.T.reshape(NK, P, NB).transpose(1, 0, 2).reshape(P, NK * NB)
        ).astype(np.float32)
        gpk = np.concatenate([clsT, w["rwt"], w["idn"], w["rep"]], axis=1)
        in_maps.append({
            "xt_in": xt,
            "gpk_in": np.ascontiguousarray(gpk),
            "d_t": w["d_t"],
            "u_in": w["u_in"],
            "idnb": w["idnb"],
        })
    return in_maps


def unshard_core(y_np):
    """y_out [NK, P*T] (bf16) -> [NB, L, H] f32."""
    y = np.asarray(y_np, np.float32).reshape(NK, P, T)
    return y.transpose(2, 0, 1).reshape(NB, L, H)


def kernel(x, router_w, lora_down, lora_up):
    nc = get_compiled()
    in_maps = make_in_maps(x, router_w, lora_down, lora_up)
    res = run_bass_kernel_spmd(nc, in_maps, core_ids=list(range(N_CORES)))
    out = np.empty((B, L, H), np.float32)
    for i in range(N_CORES):
        out[i * NB : (i + 1) * NB] = unshard_core(res.results[i]["y_out"])
    return out
